# revision 35
# baseline (speedup 1.0000x reference)
"""Trainium2 Bass kernel for nn_base_Model_40621800685765 (anomaly-transformer
encoder, B=16 L=512 D=512 H=8 DFF=2048 NL=3).

Sharding: pure data parallel -- 2 samples per core across 8 NeuronCores, no
collectives.  Activations live transposed ([D, tok] as 4x128-partition chunks)
so every matmul has its contraction dim on partitions and no activation
transposes are needed.  Matmuls run in fp32r (full PE rate at N=512).
Softmax is computed on S^T (kv on partitions): exp on ScalarE with the
1/sqrt(E) scale folded in, denominators ride a fused ones-column in the V
matmul (M=65), and the per-query 1/denom is applied via PE-broadcast planes.
LayerNorm stats use PE ones-matmuls; rsqrt = exp(-0.5*ln(var+eps)) keeps
ScalarE inside the natural_log_exp table set.  The sigma/prior branch of the
reference is dead code (DCE'd) and is not computed.
"""

import numpy as np

B, L, C = 16, 512, 38
D, H, DFF, NL, DRES = 512, 8, 2048, 3, 128
E = D // H
NCORES = 8
BPC = B // NCORES
NC_D = D // 128      # 4
NC_F = DFF // 128    # 16
NT = L // 128        # 4
SCALE = 1.0 / float(np.sqrt(E))
EPS = 1e-5

_CACHE = {}


# ---------------------------------------------------------------------------
# Walrus workaround: this neuronxcc build allows only ONE sync-wait per
# instruction.  Split any multi-wait instruction into same-engine single-wait
# NOPs + the instruction, and chain the kernel-tail drain.
# ---------------------------------------------------------------------------
def _apply_walrus_patch():
    import bass_rust
    import concourse.tile as tile
    from concourse import mybir
    from concourse._compat import not_none as nn
    from concourse.vector_clock import ScopedClock

    if getattr(tile.TileContext, "_waitsplit_patched", False):
        return

    def _split_waits(self, inst):
        si = inst.sync_info
        if (
            si is not None
            and si.on_wait is not None
            and len(si.on_wait) > 1
            and inst.engine != mybir.EngineType.Unassigned
        ):
            waits = list(si.on_wait)
            si.on_wait = waits[-1:]
            for w in waits[:-1]:
                nop = mybir.InstNoOp(
                    name=f"waitsplit-{self.nc.next_id()}",
                    sync_info=mybir.SyncInfo(on_wait=[w], on_update=[]),
                    bass_nofuse=True,
                    engine=inst.engine,
                )
                self.nc.register_instruction(nop, overwrite=True)
                nn(self.nc.cur_bb).bb.add_instruction(nop)

    def _patched_add_instruction(self, inst):
        _split_waits(self, inst)
        self.nc.register_instruction(inst, overwrite=True)
        nn(self.nc.cur_bb).bb.add_instruction(inst)

    def _patched_drain_and_barrier(self, tick_clock, wait_clock):
        nc = self.nc
        drain_inst = nc.sync.drain()
        wait_clock.add_sem_waits(
            drain_inst.ins, ScopedClock({None: tick_clock.global_clock})
        )
        si = drain_inst.ins.sync_info
        if si is not None and si.on_wait is not None and len(si.on_wait) > 1:
            waits = list(si.on_wait)
            si.on_wait = waits[:1]
            for w in waits[1:]:
                d2 = nc.sync.drain()
                d2.ins.sync_info = bass_rust.SyncInfo(on_wait=[w], on_update=[])

        nc.all_engine_barrier()
        assert self.sems is not None
        popped = nc._tile_sem_poison_stack.pop()
        assert popped is self._sem_poison
        nc.clear_and_free_semaphores(list(self.sems.allocated().values()))
        nc.all_engine_barrier()

    tile.TileContext._add_instruction = _patched_add_instruction
    tile.TileContext._drain_and_barrier = _patched_drain_and_barrier
    tile.TileContext._waitsplit_patched = True


def _build_program():
    import concourse.bass as bass
    import concourse.tile as tile
    from concourse import mybir

    _apply_walrus_patch()

    F32 = mybir.dt.float32
    F32R = mybir.dt.float32r

    nc = bass.Bass("TRN2", target_bir_lowering=False, debug=False,
                   enable_asserts=True)

    d = {}
    d["x"] = nc.dram_tensor("x", [BPC, L, C], F32R, kind="ExternalInput").ap()
    d["emb_kernel"] = nc.dram_tensor("emb_kernel", [D, C, 3], F32R, kind="ExternalInput").ap()
    for n, sh, dt_ in [
        ("Wq", [NL, D, D], F32R), ("Wk", [NL, D, D], F32R),
        ("Wv", [NL, D, D], F32R), ("Wo", [NL, D, D], F32R),
        ("W1", [NL, D, DFF], F32R), ("W2", [NL, DFF, D], F32R),
        ("bq", [NL, D], F32), ("bk", [NL, D], F32),
        ("bv", [NL, D], F32R), ("bo", [NL, D], F32R),
        ("b1", [NL, DFF], F32), ("b2", [NL, D], F32R),
        ("n1g", [NL, D], F32), ("n1b", [NL, D], F32),
        ("n2g", [NL, D], F32), ("n2b", [NL, D], F32),
        ("nfg", [D], F32), ("nfb", [D], F32),
        ("Wproj", [D, D], F32R), ("bproj", [D], F32R),
        ("Wfea", [DRES, D], F32R), ("bfea", [D], F32R),
        ("ident", [128, 128], F32R),
        ("selmat", [2, 128], F32R),
        ("onesmat", [128, 512], F32R),
        ("zeromat", [32, 512], F32R),
    ]:
        d[n] = nc.dram_tensor(n, sh, dt_, kind="ExternalInput").ap()

    d["o_fea"] = nc.dram_tensor("fea_t", [BPC, L, D], F32, kind="ExternalOutput").ap()
    d["o_res"] = nc.dram_tensor("enc_res", [BPC, L, D - DRES], F32, kind="ExternalOutput").ap()
    d["o_rec"] = nc.dram_tensor("rec_temp", [BPC, L, D], F32, kind="ExternalOutput").ap()
    d["o_emb"] = nc.dram_tensor("emb", [BPC, L, D], F32, kind="ExternalOutput").ap()

    with tile.TileContext(nc) as tc:
        _emit(nc, tc, mybir, d)
    return nc


def _emit(nc, tc, mybir, d):
    from contextlib import ExitStack

    F32 = mybir.dt.float32
    F32R = mybir.dt.float32r
    AF = mybir.ActivationFunctionType
    ALU = mybir.AluOpType

    with ExitStack() as ctx:
        tp = lambda name, bufs, **kw: ctx.enter_context(
            tc.tile_pool(name=name, bufs=bufs, **kw))
        consts = tp("consts", 1)
        wp = tp("wp", 17)        # Wq/Wk/Wv/Wo per-kc [128,512] tiles
        w1p = tp("w1p", 6)       # W1 per-dc [128,4,128], reloaded per sample
        w2p = tp("w2p", 4)       # W2 per-dc [128,512], reloaded per sample
        hp = tp("hp", 8)        # hT / enc chunks
        qp = tp("qp", 4)
        kp = tp("kp", 4)
        vp = tp("vp", 4)         # v65 [128, 8, 65]
        atp = tp("atp", 4)       # assembled attnT chunks (unnorm f32 + norm f32r)
        tmp64p = tp("tmp64p", 1)
        xp = tp("xp", 4)         # x1T
        zp = tp("zp", 4)         # residual z
        sqp = tp("sqp", 3)       # squares + ln tmp
        ep = tp("ep", 3)         # expST kv-pairs [128,1024]
        a1p = tp("a1p", 3)       # gelu out + xcat/xT staging
        smp = tp("smp", 2)       # scr, denpack, xn
        stp = tp("stp", 1)       # LN stats row
        outp = tp("outp", 2)     # output staging
        psA = tp("psA", 2, space="PSUM")     # 1-bank rotation (acc, pa, ps_g, pb)
        psB2 = tp("psB2", 2, space="PSUM")   # 2-bank tiles: score pairs / FFN acc
        psM = tp("psM", 2, space="PSUM")     # stats rows + bcast planes

        # ---- constants ---------------------------------------------------
        ones_col = consts.tile([128, 1], F32R, tag="c_onescol")
        nc.sync.dma_start(ones_col, d["onesmat"][:, 0:1])
        ones_v = consts.tile([128, H], F32R, tag="c_onesv")
        nc.sync.dma_start(ones_v, d["onesmat"][:, 0:H])
        eps_t = consts.tile([128, 1], F32, tag="c_eps")
        nc.vector.memset(eps_t, EPS)
        ident = consts.tile([128, 128], F32R, tag="c_ident")
        nc.sync.dma_start(ident, d["ident"][:, :])
        # pair selector [2,128]: row 0 -> cols 0:64, row 1 -> cols 64:128
        sel2 = consts.tile([2, 128], F32R, tag="c_sel2")
        nc.sync.dma_start(sel2, d["selmat"][:, :])

        def pp(dram_row, tag):   # [D] -> [128, NC_D] per-partition params
            tl = consts.tile([128, NC_D], F32, tag=tag)
            nc.sync.dma_start(tl, dram_row.rearrange("(c p) -> p c", p=128))
            return tl

        n1g = [pp(d["n1g"][l], f"c_n1g{l}") for l in range(NL)]
        n1b = [pp(d["n1b"][l], f"c_n1b{l}") for l in range(NL)]
        n2g = [pp(d["n2g"][l], f"c_n2g{l}") for l in range(NL)]
        n2b = [pp(d["n2b"][l], f"c_n2b{l}") for l in range(NL)]
        nfg = pp(d["nfg"], "c_nfg")
        nfb = pp(d["nfb"], "c_nfb")
        bq = [pp(d["bq"][l], f"c_bq{l}") for l in range(NL)]
        bk = [pp(d["bk"][l], f"c_bk{l}") for l in range(NL)]
        b1 = []
        for l in range(NL):
            tl = consts.tile([128, NC_F], F32, tag=f"c_b1_{l}")
            nc.sync.dma_start(tl, d["b1"][l].rearrange("(c p) -> p c", p=128))
            b1.append(tl)
        # bias rows as matmul operands need base_partition in {0,32,64}:
        # pack 3 rows per [65, D] tile at partitions 0/32/64.
        # order: bv[0..2], b2[0..2], bo[0..2], bproj, bfea
        browpack = [consts.tile([65, D], F32R, tag=f"c_brow{i}", name=f"brow{i}")
                    for i in range(4)]
        # ones rows at bases 0/32/64 to pair with the packed bias rows
        ones512p = consts.tile([65, D], F32R, tag="c_ones512p")
        nc.sync.dma_start(ones512p, d["onesmat"][0:65, :])
        ones128p = consts.tile([65, 128], F32R, tag="c_ones128p")
        nc.sync.dma_start(ones128p, d["onesmat"][0:65, 0:128])

        def _brow(idx):
            return browpack[idx // 3][32 * (idx % 3):32 * (idx % 3) + 1, :]

        def _bbase(idx):
            return 32 * (idx % 3)

        def ones_row_at(b):
            return ones512p[b:b + 1, :]

        def ones128_at(b):
            return ones128p[b:b + 1, :]

        for l in range(NL):
            nc.sync.dma_start(_brow(l), d["bv"][l][None, :])
            nc.sync.dma_start(_brow(3 + l), d["b2"][l][None, :])
            nc.sync.dma_start(_brow(6 + l), d["bo"][l][None, :])
        nc.sync.dma_start(_brow(9), d["bproj"][None, :])
        nc.sync.dma_start(_brow(10), d["bfea"][None, :])
        bv_row = lambda l: _brow(l)
        b2_row = lambda l: _brow(3 + l)
        bo_row = lambda l: _brow(6 + l)

        wembA = consts.tile([102, D], F32R, tag="c_wembA")
        nc.sync.dma_start(wembA[0:38, :],
                          d["emb_kernel"].rearrange("d c k -> k c d")[0])
        nc.sync.dma_start(wembA[38:64, :], d["zeromat"][0:26, :])
        nc.sync.dma_start(wembA[64:102, :],
                          d["emb_kernel"].rearrange("d c k -> k c d")[1])
        wembB = consts.tile([38, D], F32R, tag="c_wembB")
        nc.sync.dma_start(wembB[:, :],
                          d["emb_kernel"].rearrange("d c k -> k c d")[2])
        wproj = consts.tile([128, NC_D, D], F32R, tag="c_wproj")
        for kc in range(NC_D):
            nc.sync.dma_start(wproj[:, kc, :], d["Wproj"][kc * 128:(kc + 1) * 128, :])
        wfea = consts.tile([128, D], F32R, tag="c_wfea")
        nc.sync.dma_start(wfea, d["Wfea"][:, :])

        # ---- embedding ---------------------------------------------------
        h = {}
        xcatA, xcatB = {}, {}
        for s in range(BPC):
            xcatA[s] = a1p.tile([102, L], F32R, tag="a1", name=f"xcatA{s}")
            xcatB[s] = vp.tile([38, L], F32R, tag="v", name=f"xcatB{s}")
            nc.gpsimd.dma_start(xcatA[s][38:64, :], d["zeromat"][0:26, 0:L])
        for tt in range(NT):
            for s in range(BPC):
                xn = smp.tile([128, C], F32R, tag="xn")
                nc.gpsimd.dma_start(xn, d["x"][s][tt * 128:(tt + 1) * 128, :])
                ps0 = psA.tile([128, 128], F32R, tag="mm")
                nc.tensor.transpose(ps0[0:C, 0:128], xn, ident)
                lo = tt * 128
                stg = smp.tile([38, 128], F32R, tag="xstg")
                nc.vector.tensor_copy(stg, ps0[0:C, 0:128])
                nc.gpsimd.dma_start(xcatA[s][64:64 + C, lo:lo + 128], stg)
                if tt < NT - 1:
                    nc.vector.tensor_copy(xcatA[s][0:C, lo + 1:lo + 129],
                                          ps0[0:C, 0:128])
                else:
                    nc.vector.tensor_copy(xcatA[s][0:C, lo + 1:L], ps0[0:C, 0:127])
                    nc.vector.tensor_copy(xcatA[s][0:C, 0:1], ps0[0:C, 127:128])
                if tt > 0:
                    nc.vector.tensor_copy(xcatB[s][0:C, lo - 1:lo + 127],
                                          ps0[0:C, 0:128])
                else:
                    nc.vector.tensor_copy(xcatB[s][0:C, 0:127], ps0[0:C, 1:128])
                    nc.vector.tensor_copy(xcatB[s][0:C, L - 1:L], ps0[0:C, 0:1])
        for s in range(BPC):
            for oc in range(NC_D):
                ps = psA.tile([128, L], F32, tag="mm")
                nc.tensor.matmul(ps, wembA[:, oc * 128:(oc + 1) * 128], xcatA[s],
                                 start=True, stop=False)
                nc.tensor.matmul(ps, wembB[:, oc * 128:(oc + 1) * 128], xcatB[s],
                                 start=False, stop=True)
                ht = hp.tile([128, L], F32R, tag="h")
                nc.vector.tensor_copy(ht, ps)
                h[(s, oc)] = ht
            for tt in range(NT):
                ps = psA.tile([128, D], F32, tag="mm")
                nc.tensor.matmul(ps, xcatA[s][:, tt * 128:(tt + 1) * 128], wembA,
                                 start=True, stop=False)
                nc.tensor.matmul(ps, xcatB[s][:, tt * 128:(tt + 1) * 128], wembB,
                                 start=False, stop=True)
                ob = outp.tile([128, D], F32, tag="out")
                nc.vector.tensor_copy(ob, ps)
                nc.gpsimd.dma_start(d["o_emb"][s][tt * 128:(tt + 1) * 128, :], ob)

        # ---- split T-layout LayerNorm: ln_a (sums+stats) / ln_b (apply) ----
        def ln_a(zt, g, b):
            ps1 = psM.tile([1, L], F32, tag="sm")
            ps2 = psM.tile([1, L], F32, tag="sm")
            for c in range(NC_D):
                nc.tensor.matmul(ps1, ones_col, zt[c],
                                 start=(c == 0), stop=(c == NC_D - 1))
            sq = []
            for c in range(NC_D):
                sqt = sqp.tile([128, L], F32R, tag="sq")
                nc.vector.tensor_mul(sqt, zt[c].bitcast(F32), zt[c].bitcast(F32))
                sq.append(sqt)
            for c in range(NC_D):
                nc.tensor.matmul(ps2, ones_col, sq[c],
                                 start=(c == 0), stop=(c == NC_D - 1))
            # stats on partition-0 row segments; runs on DVE/ACT under the
            # following dense PE phase
            st = stp.tile([1, 3 * L], F32R, tag="st")
            stf = st.bitcast(F32)
            sm = lambda i: st[0:1, i * L:(i + 1) * L]
            smf = lambda i: stf[0:1, i * L:(i + 1) * L]
            nc.vector.tensor_scalar_mul(sm(0), ps1, 1.0 / D)       # m
            nc.vector.tensor_scalar_mul(sm(1), ps2, 1.0 / D)       # E[x2]
            nc.vector.tensor_mul(sm(2), smf(0), smf(0))            # m^2
            nc.vector.tensor_sub(sm(1), smf(1), smf(2))            # var
            nc.scalar.activation(sm(2), smf(1), AF.Ln,
                                 bias=eps_t[0:1, :], scale=1.0)
            nc.scalar.activation(sm(2), smf(2), AF.Exp, scale=-0.5)  # rs
            nc.vector.tensor_mul(sm(1), smf(0), smf(2))            # m*rs
            return (zt, st, g, b)

        def ln_b(state, out_pool, out_tag):
            zt, st, g, b = state
            sm = lambda i: st[0:1, i * L:(i + 1) * L]
            pb_rs = psM.tile([128, L], F32, tag="sm")
            pb_mrs = psM.tile([128, L], F32, tag="sm")
            nc.tensor.matmul(pb_rs, ones128_at(0), sm(2), start=True, stop=True)
            nc.tensor.matmul(pb_mrs, ones128_at(0), sm(1), start=True, stop=True)
            out = []
            for c in range(NC_D):
                tmp = sqp.tile([128, L], F32, tag="lntmp")
                nc.vector.tensor_mul(tmp, zt[c].bitcast(F32), pb_rs)
                nc.vector.tensor_sub(tmp, tmp, pb_mrs)
                o = out_pool.tile([128, L], F32R, tag=out_tag, name=f"ln{c}")
                nc.vector.tensor_scalar(out=o, in0=tmp,
                                        scalar1=g[:, c:c + 1],
                                        scalar2=b[:, c:c + 1],
                                        op0=ALU.mult, op1=ALU.add)
                out.append(o)
            return out

        # ---- per-phase emitters -----------------------------------------
        def qkv_phase(s, l, wq, wk, wv):
            q, k, v65 = [], [], []
            for oc in range(NC_D):
                ps = psA.tile([128, L], F32, tag="mm")
                for kc in range(NC_D):
                    nc.tensor.matmul(ps, wq[kc][:, oc * 128:(oc + 1) * 128],
                                     h[(s, kc)], start=(kc == 0),
                                     stop=(kc == NC_D - 1))
                qt = qp.tile([128, L], F32R, tag="q")
                nc.vector.tensor_scalar_add(qt, ps, bq[l][:, oc:oc + 1])
                q.append(qt)
            for oc in range(NC_D):
                ps = psA.tile([128, L], F32, tag="mm")
                for kc in range(NC_D):
                    nc.tensor.matmul(ps, wk[kc][:, oc * 128:(oc + 1) * 128],
                                     h[(s, kc)], start=(kc == 0),
                                     stop=(kc == NC_D - 1))
                kt = kp.tile([128, L], F32R, tag="k")
                nc.vector.tensor_scalar_add(kt, ps, bk[l][:, oc:oc + 1])
                k.append(kt)
            for tt in range(NT):
                ps = psA.tile([128, D], F32, tag="mm")
                for kc in range(NC_D):
                    nc.tensor.matmul(ps, h[(s, kc)][:, tt * 128:(tt + 1) * 128],
                                     wv[kc], start=(kc == 0), stop=False)
                nc.tensor.matmul(ps, ones128_at(_bbase(l)), bv_row(l),
                                 start=False, stop=True)
                vt = vp.tile([128, H, E + 1], F32R, tag="v")
                nc.vector.tensor_copy(vt[:, :, E:E + 1],
                                      ones_v.rearrange("p (h o) -> p h o", o=1))
                nc.vector.tensor_copy(vt[:, :, 0:E],
                                      ps.rearrange("p (h e) -> p h e", h=H))
                v65.append(vt)
            return q, k, v65

        def attn_phase(qkv):
            q, k, v65 = qkv
            # phase 1: all heads' scores/exp/attnV + denominator staging;
            # phase 2: per-pair normalization (denominator chain latency is
            # covered by the remaining heads' matmuls)
            ats, recs = [], []
            for hp_c in range(NC_D):
                at = atp.tile([128, L], F32, tag="at")
                denpk2 = smp.tile([2, L], F32, tag="denpack")
                for h2 in range(2):
                    hd = 2 * hp_c + h2
                    r0 = h2 * 64
                    est = []
                    for kvp in range(2):
                        ps_s2 = psB2.tile([128, 2 * L], F32, tag="b2")
                        for j in range(2):
                            kv = 2 * kvp + j
                            nc.tensor.matmul(
                                ps_s2[:, j * L:(j + 1) * L],
                                k[hp_c][r0:r0 + 64, kv * 128:(kv + 1) * 128],
                                q[hp_c][r0:r0 + 64, :], start=True, stop=True)
                        ex = ep.tile([128, 2 * L], F32R, tag="exp")
                        nc.scalar.activation(ex, ps_s2, AF.Exp, scale=SCALE)
                        est.append(ex)
                    pa = psA.tile([65, L], F32, tag="mm")
                    for kv in range(NT):
                        nc.tensor.matmul(
                            pa, v65[kv][:, hd, :],
                            est[kv // 2][:, (kv % 2) * L:(kv % 2 + 1) * L],
                            start=(kv == 0), stop=(kv == NT - 1))
                    scr = smp.tile([65, L], F32, tag="scr")
                    nc.vector.tensor_copy(scr[64:65, :], pa[64:65, :])
                    nc.sync.dma_start(denpk2[h2:h2 + 1, :], scr[64:65, :])
                    if h2 == 0:
                        nc.vector.tensor_copy(at[0:64, :], pa[0:64, :])
                    else:
                        t64 = tmp64p.tile([64, L], F32, tag="t64")
                        nc.vector.tensor_copy(t64, pa[0:64, :])
                        nc.sync.dma_start(at[64:128, :], t64)
                denrec2 = smp.tile([2, L], F32R, tag="denrec")
                with nc.allow_low_precision(reason="f32r softmax denom"):
                    nc.vector.reciprocal(denrec2, denpk2)
                ats.append(at)
                recs.append(denrec2)
            attn = []
            for hp_c in range(NC_D):
                pb = psM.tile([128, L], F32, tag="sm")
                nc.tensor.matmul(pb, sel2, recs[hp_c], start=True, stop=True)
                pb_sb = sqp.tile([128, L], F32, tag="lntmp")
                nc.vector.tensor_copy(pb_sb, pb)
                atn = atp.tile([128, L], F32R, tag="atn")
                nc.vector.tensor_mul(atn, ats[hp_c], pb_sb)
                attn.append(atn)
            return attn

        def wo_res(s, l, wo, attn):
            z1 = []
            for oc in range(NC_D):
                ps = psA.tile([128, L], F32, tag="mm")
                for kc in range(NC_D):
                    nc.tensor.matmul(ps, wo[kc][:, oc * 128:(oc + 1) * 128],
                                     attn[kc], start=(kc == 0), stop=False)
                nc.tensor.matmul(ps, bo_row(l)[:, oc * 128:(oc + 1) * 128],
                                 ones_row_at(_bbase(6 + l)),
                                 start=False, stop=True)
                zt = zp.tile([128, L], F32R, tag="z")
                nc.vector.tensor_add(zt, ps, h[(s, oc)].bitcast(F32))
                z1.append(zt)
            return z1

        def ffn_phase(l, x1):
            # two 2-bank accumulators hold the 4 output chunks; W2(dc-2) runs
            # under W1(dc) so gelu latency never stalls PE; weight loads
            # alternate between the two DMA paths.
            ps_y2 = [psB2.tile([128, 2 * L], F32, tag="b2", name=f"psy2_{i_}")
                     for i_ in range(2)]

            def psy(oc):
                return ps_y2[oc // 2][:, (oc % 2) * L:(oc % 2 + 1) * L]

            def w2mm(a1, w2t, dc):
                for oc in range(NC_D):
                    nc.tensor.matmul(psy(oc),
                                     w2t[:, oc * 128:(oc + 1) * 128], a1,
                                     start=(dc == 0), stop=False)

            pipe = []
            for dc in range(NC_F):
                eng1 = nc.sync if dc % 2 == 0 else nc.gpsimd
                eng2 = nc.gpsimd if dc % 2 == 0 else nc.sync
                w1t = w1p.tile([128, NC_D, 128], F32R, tag="w1")
                for kc in range(NC_D):
                    eng1.dma_start(
                        w1t[:, kc, :],
                        d["W1"][l][kc * 128:(kc + 1) * 128,
                                   dc * 128:(dc + 1) * 128])
                w2t = w2p.tile([128, D], F32R, tag="w2")
                eng2.dma_start(w2t, d["W2"][l][dc * 128:(dc + 1) * 128, :])
                ps_g = psA.tile([128, L], F32, tag="mm")
                for kc in range(NC_D):
                    nc.tensor.matmul(ps_g, w1t[:, kc, :], x1[kc],
                                     start=(kc == 0), stop=(kc == NC_D - 1))
                if len(pipe) >= 2:
                    w2mm(*pipe.pop(0))
                a1 = a1p.tile([128, L], F32R, tag="a1")
                nc.scalar.activation(a1, ps_g, AF.Gelu,
                                     bias=b1[l][:, dc:dc + 1], scale=1.0)
                pipe.append((a1, w2t, dc))
            while pipe:
                w2mm(*pipe.pop(0))
            z2 = []
            for oc in range(NC_D):
                nc.tensor.matmul(psy(oc),
                                 b2_row(l)[:, oc * 128:(oc + 1) * 128],
                                 ones_row_at(_bbase(3 + l)),
                                 start=False, stop=True)
                zt = zp.tile([128, L], F32R, tag="z")
                nc.vector.tensor_add(zt, psy(oc), x1[oc].bitcast(F32))
                z2.append(zt)
            return z2

        # ---- layers: two-sample software pipeline ------------------------
        pend_ln2 = None
        for l in range(NL):
            wq, wk, wv, wo = [], [], [], []
            for name, lst in [("Wq", wq), ("Wk", wk), ("Wv", wv), ("Wo", wo)]:
                for kc in range(NC_D):
                    w = wp.tile([128, D], F32R, tag="wp")
                    nc.sync.dma_start(w, d[name][l][kc * 128:(kc + 1) * 128, :])
                    lst.append(w)

            qkv0 = qkv_phase(0, l, wq, wk, wv)
            if pend_ln2 is not None:
                hn = ln_b(pend_ln2, hp, "h")
                for oc in range(NC_D):
                    h[(1, oc)] = hn[oc]
                pend_ln2 = None
            attn0 = attn_phase(qkv0)
            qkv1 = qkv_phase(1, l, wq, wk, wv)
            z1_0 = wo_res(0, l, wo, attn0)
            st1_0 = ln_a(z1_0, n1g[l], n1b[l])
            attn1 = attn_phase(qkv1)
            x1_0 = ln_b(st1_0, xp, "x1")
            z1_1 = wo_res(1, l, wo, attn1)
            st1_1 = ln_a(z1_1, n1g[l], n1b[l])
            z2_0 = ffn_phase(l, x1_0)
            x1_1 = ln_b(st1_1, xp, "x1")
            st2_0 = ln_a(z2_0, n2g[l], n2b[l])
            z2_1 = ffn_phase(l, x1_1)
            hn0 = ln_b(st2_0, hp, "h")
            for oc in range(NC_D):
                h[(0, oc)] = hn0[oc]
            pend_ln2 = ln_a(z2_1, n2g[l], n2b[l])

        # ---- final: LNf + outputs (pipelined) ---------------------------
        def emit_outputs(s, enc):
            for c in range(3):
                for tt in range(NT):
                    ps = psA.tile([128, 128], F32R, tag="mm")
                    nc.tensor.transpose(ps, enc[c][:, tt * 128:(tt + 1) * 128],
                                        ident)
                    ob = a1p.tile([128, D], F32, tag="a1", name="obr")
                    nc.vector.tensor_copy(ob[:, 0:128], ps)
                    nc.gpsimd.dma_start(
                        d["o_res"][s][tt * 128:(tt + 1) * 128,
                                      c * 128:(c + 1) * 128], ob[:, 0:128])
            for tt in range(NT):
                ps = psA.tile([128, D], F32, tag="mm")
                nc.tensor.matmul(ps, enc[3][:, tt * 128:(tt + 1) * 128], wfea,
                                 start=True, stop=False)
                nc.tensor.matmul(ps, ones128_at(32), _brow(10),
                                 start=False, stop=True)
                ob = outp.tile([128, D], F32, tag="out")
                nc.vector.tensor_copy(ob, ps)
                nc.gpsimd.dma_start(d["o_fea"][s][tt * 128:(tt + 1) * 128, :], ob)
            for tt in range(NT):
                ps = psA.tile([128, D], F32, tag="mm")
                for kc in range(NC_D):
                    nc.tensor.matmul(ps, enc[kc][:, tt * 128:(tt + 1) * 128],
                                     wproj[:, kc, :], start=(kc == 0), stop=False)
                nc.tensor.matmul(ps, ones128_at(0), _brow(9),
                                 start=False, stop=True)
                ob = outp.tile([128, D], F32, tag="out")
                nc.vector.tensor_copy(ob, ps)
                nc.gpsimd.dma_start(d["o_rec"][s][tt * 128:(tt + 1) * 128, :], ob)

        hn = ln_b(pend_ln2, hp, "h")
        for oc in range(NC_D):
            h[(1, oc)] = hn[oc]
        stf0 = ln_a([h[(0, c)] for c in range(NC_D)], nfg, nfb)
        enc0 = ln_b(stf0, hp, "h")
        stf1 = ln_a([h[(1, c)] for c in range(NC_D)], nfg, nfb)
        emit_outputs(0, enc0)
        enc1 = ln_b(stf1, hp, "h")
        emit_outputs(1, enc1)


def kernel(**inputs):
    from concourse import bass_utils

    if "nc" not in _CACHE:
        _CACHE["nc"] = _build_program()
    nc = _CACHE["nc"]

    f32 = lambda a: np.ascontiguousarray(np.asarray(a), dtype=np.float32)
    shared_names = ["emb_kernel", "Wq", "Wk", "Wv", "Wo", "W1", "W2",
                    "bq", "bk", "bv", "bo", "b1", "b2",
                    "n1g", "n1b", "n2g", "n2b", "nfg", "nfb",
                    "Wproj", "bproj", "Wfea", "bfea"]
    shared = {n: f32(inputs[n]) for n in shared_names}
    shared["ident"] = np.eye(128, dtype=np.float32)
    selmat = np.zeros((2, 128), dtype=np.float32)
    selmat[0, 0:64] = 1.0
    selmat[1, 64:128] = 1.0
    shared["selmat"] = selmat
    shared["onesmat"] = np.ones((128, 512), dtype=np.float32)
    shared["zeromat"] = np.zeros((32, 512), dtype=np.float32)
    x = f32(inputs["x"])

    in_maps = []
    for c in range(NCORES):
        m = dict(shared)
        m["x"] = np.ascontiguousarray(x[c * BPC:(c + 1) * BPC])
        in_maps.append(m)

    res = bass_utils.run_bass_kernel_spmd(nc, in_maps, core_ids=list(range(NCORES)))
    fea = np.concatenate([r["fea_t"] for r in res.results], axis=0)
    enc_res = np.concatenate([r["enc_res"] for r in res.results], axis=0)
    rec = np.concatenate([r["rec_temp"] for r in res.results], axis=0)
    emb = np.concatenate([r["emb"] for r in res.results], axis=0)
    return fea, enc_res, rec, emb
